# revision 14
# baseline (speedup 1.0000x reference)
"""Trainium2 Bass kernel for a custom GRU cell.

    x_h   = concat([inputs, h_prev], -1)            # [B, D+U]
    z     = sigmoid(x_h @ Wz)                       # [B, U]
    r     = sigmoid(x_h @ Wr)                       # [B, U]
    h_hat = tanh(concat([inputs, r * h_prev]) @ Wh) # [B, U]
    out   = z * h_prev + (1 - z) * h_hat

Data-parallel over 8 NeuronCores: batch sharded, weights replicated.

Per-core (B_c = 2048 rows, processed as 4 blocks of 512):
  - all matmuls in f32r (fp32 HIGH-half mode: ~bf16 speed, ~2^-13 precision)
  - x_h is transposed on the PE (f32r transpose-mode, 128x128 blocks) into
    feature-major k-tiles xh[k] [128, 512-batch], staged through paired
    2-bank PSUM tiles and copied to SBUF by ScalarE
  - gate z batch-major:  psum[b,u]  = xh[k][:,bslice].T @ Wz[k]
  - gate r TRANSPOSED:   psum[u,b]  = Wr[k][:,uslice].T @ xh[k]
    so r*h_prev is computed transposed (rT * hT, where hT = xh[4+u]) with
    no extra transposes, and feeds gate h as lhsT directly
  - gate h batch-major, tanh/sigmoid on ScalarE straight out of PSUM,
    combine on VectorE, DMA out
  - emission is software-pipelined: gate-h of block i after z/r of i+1
"""

import sys

for _p in ("/opt/trn_rl_repo", "/root/.axon_site/_ro/trn_rl_repo"):
    if _p not in sys.path:
        sys.path.append(_p)

import numpy as np

B, D, U = 16384, 512, 512
K = D + U
N_CORES = 8
BC = B // N_CORES          # rows per core (2048)
BB = 512                   # batch-block rows
NB = BC // BB              # blocks per core (4)
KC = K // 128              # contraction chunks (8)


def build_gru_tile_kernel(tc, d_in, d_hp, d_wz, d_wr, d_wh, d_out, nb=NB):
    """Emit the GRU cell body into TileContext `tc`."""
    import contextlib

    from concourse import mybir
    from concourse.masks import make_identity

    f32 = mybir.dt.float32
    f32r = mybir.dt.float32r
    nc = tc.nc
    Sig = mybir.ActivationFunctionType.Sigmoid
    Tanh = mybir.ActivationFunctionType.Tanh

    est = contextlib.ExitStack()
    sing = est.enter_context(tc.tile_pool(name="sing", bufs=1))
    wpool = est.enter_context(tc.tile_pool(name="w", bufs=1))
    io = est.enter_context(tc.tile_pool(name="io", bufs=8))
    hpool = est.enter_context(tc.tile_pool(name="hpool", bufs=8))
    xhp = est.enter_context(tc.tile_pool(name="xhp", bufs=16))
    rhp = est.enter_context(tc.tile_pool(name="rhp", bufs=6))
    actp = est.enter_context(tc.tile_pool(name="act", bufs=6))
    tmpp = est.enter_context(tc.tile_pool(name="tmp", bufs=4))
    # PSUM: 3 transpose staging banks + 5 gate banks = 8/8
    pst = est.enter_context(tc.tile_pool(name="pst", bufs=2, space="PSUM"))
    psg = est.enter_context(tc.tile_pool(name="psg", bufs=6, space="PSUM"))

    ident0 = sing.tile([128, 128], f32)
    make_identity(nc, ident0)
    identr = sing.tile([128, 128], f32r)
    nc.scalar.copy(identr[:], ident0[:])

    # ---- DMA schedule ----
    # The DMA engines drain instructions roughly in issue order, so load
    # block 0/1 activations first (unblocks the PE transposes ~12us in),
    # then stream the weights per-chunk (each z/r/h matmul only waits on
    # its own chunk), interleaved with the remaining blocks.
    pre_x = {}
    def load_x(bb):
        xin, hps = [], []
        for j in range(4):
            r0 = bb * BB + 128 * j
            x_j = io.tile([128, 512], f32r, tag="xin", name=f"x_{bb}_{j}")
            nc.sync.dma_start(x_j[:], d_in[r0:r0 + 128, :].bitcast(f32r))
            xin.append(x_j)
            h_j = hpool.tile([128, 512], f32r, tag="hp", name=f"h_{bb}_{j}")
            nc.sync.dma_start(h_j[:], d_hp[r0:r0 + 128, :].bitcast(f32r))
            hps.append(h_j)
        pre_x[bb] = (xin, hps)

    w_sb = {}
    def load_w(name, dram):
        t = wpool.tile([128, KC, 512], f32r, tag=name, name=name)
        for k in range(KC):
            nc.sync.dma_start(t[:, k, :], dram[128 * k:128 * (k + 1), :].bitcast(f32r))
        w_sb[name] = t

    load_x(0)
    load_w("wz", d_wz)
    load_w("wr", d_wr)
    if nb > 1:
        load_x(1)
    load_w("wh", d_wh)
    for bb in range(2, nb):
        load_x(bb)

    state = [None] * nb

    def phase_zr(bb):
        xin, hps = pre_x[bb]

        # ---- PE-transpose into feature-major k-tiles xh[k] [128, 512b] ----
        # The z-gate accumulation is split around the h_prev transposes so
        # the PE has matmuls in flight while the last transpose copies land.
        xh = [None] * KC
        xh_tiles = []

        def transpose_group(k):
            ps1 = pst.tile([128, 512], f32r, tag="pst", name=f"pst_{bb}_{k}")
            src = xin if k < 4 else hps
            kk = k % 4
            for j in range(4):
                nc.tensor.transpose(ps1[:, 128 * j:128 * (j + 1)],
                                    src[j][:, 128 * kk:128 * (kk + 1)], identr[:])
            sb1 = xhp.tile([128, 512], f32r, tag="xh", name=f"xh_{bb}_{k}")
            nc.scalar.copy(sb1[:], ps1[:])
            xh_tiles.append(sb1)
            xh[k] = sb1[:]

        for k in range(KC):
            transpose_group(k)

        # gate z, batch-major: ps[b,u] += xh[k][:,j].T @ Wz[k]
        zs = []
        for j in range(4):
            ps = psg.tile([128, 512], f32, tag="psg", name=f"psz_{bb}_{j}")
            for k in range(KC):
                nc.tensor.matmul(ps[:], xh[k][:, 128 * j:128 * (j + 1)],
                                 w_sb["wz"][:, k, :],
                                 start=(k == 0), stop=(k == KC - 1))
            z_j = actp.tile([128, 512], f32, tag="z", name=f"z_{bb}_{j}")
            nc.scalar.activation(z_j[:], ps[:], Sig)
            zs.append(z_j)

        # ---- gate r, transposed: ps[u,b] += Wr[k][:,u].T @ xh[k] ----
        rhT = []
        for u in range(4):
            ps = psg.tile([128, 512], f32, tag="psg", name=f"psr_{bb}_{u}")
            for k in range(KC):
                nc.tensor.matmul(ps[:], w_sb["wr"][:, k, 128 * u:128 * (u + 1)],
                                 xh[k], start=(k == 0), stop=(k == KC - 1))
            rT_u = actp.tile([128, 512], f32, tag="rT", name=f"rT_{bb}_{u}")
            nc.scalar.activation(rT_u[:], ps[:], Sig)
            # rhT[u] = rT[u] * h_prev.T[u]  (hT = xh[4+u]), f32r out
            rh_u = rhp.tile([128, 512], f32r, tag="rhT", name=f"rh_{bb}_{u}")
            nc.vector.tensor_mul(rh_u[:], rT_u[:], xh[4 + u].bitcast(f32))
            rhT.append(rh_u)

        state[bb] = (xh, xh_tiles, hps, zs, rhT)

    def phase_h(bb):
        xh, xh_tiles, hps, zs, rhT = state[bb]
        for j in range(4):
            ps = psg.tile([128, 512], f32, tag="psg", name=f"psh_{bb}_{j}")
            for k in range(KC):
                lhs = (xh[k][:, 128 * j:128 * (j + 1)] if k < 4
                       else rhT[k - 4][:, 128 * j:128 * (j + 1)])
                nc.tensor.matmul(ps[:], lhs, w_sb["wh"][:, k, :],
                                 start=(k == 0), stop=(k == KC - 1))
            hh = actp.tile([128, 512], f32, tag="hh", name=f"hh_{bb}_{j}")
            nc.scalar.activation(hh[:], ps[:], Tanh)

            # out = hh + z * (hp - hh)
            hp_f = hps[j][:].bitcast(f32)
            t = tmpp.tile([128, 512], f32, tag="tmp", name=f"t_{bb}_{j}")
            nc.vector.tensor_sub(t[:], hp_f, hh[:])
            t2 = tmpp.tile([128, 512], f32, tag="tmp", name=f"t2_{bb}_{j}")
            nc.vector.tensor_mul(t2[:], zs[j][:], t[:])
            out = tmpp.tile([128, 512], f32, tag="out", name=f"o_{bb}_{j}")
            nc.vector.tensor_add(out[:], t2[:], hh[:])
            r0 = bb * BB + 128 * j
            nc.sync.dma_start(d_out[r0:r0 + 128, :], out[:])
        state[bb] = None

    phase_zr(0)
    for bb in range(1, nb):
        phase_zr(bb)
        phase_h(bb - 1)
    phase_h(nb - 1)

    est.close()


_NC_CACHE = {}


def _build(nb=NB):
    if nb in _NC_CACHE:
        return _NC_CACHE[nb]
    import concourse.tile as tile
    from concourse import bacc, mybir

    f32 = mybir.dt.float32
    nc = bacc.Bacc("TRN2", target_bir_lowering=False, debug=False)
    d_in = nc.dram_tensor("inputs", [nb * BB, D], f32, kind="ExternalInput").ap()
    d_hp = nc.dram_tensor("h_prev", [nb * BB, U], f32, kind="ExternalInput").ap()
    d_wz = nc.dram_tensor("Wz", [K, U], f32, kind="ExternalInput").ap()
    d_wr = nc.dram_tensor("Wr", [K, U], f32, kind="ExternalInput").ap()
    d_wh = nc.dram_tensor("Wh", [K, U], f32, kind="ExternalInput").ap()
    d_out = nc.dram_tensor("out", [nb * BB, U], f32, kind="ExternalOutput").ap()

    with tile.TileContext(nc) as tc:
        build_gru_tile_kernel(tc, d_in, d_hp, d_wz, d_wr, d_wh, d_out, nb=nb)
    nc.compile()
    _NC_CACHE[nb] = nc
    return nc


def run_sharded(inputs, h_prev, Wz, Wr, Wh, trace=False):
    from concourse.bass_utils import run_bass_kernel_spmd

    nc = _build()
    inputs = np.ascontiguousarray(np.asarray(inputs, dtype=np.float32))
    h_prev = np.ascontiguousarray(np.asarray(h_prev, dtype=np.float32))
    Wz = np.ascontiguousarray(np.asarray(Wz, dtype=np.float32))
    Wr = np.ascontiguousarray(np.asarray(Wr, dtype=np.float32))
    Wh = np.ascontiguousarray(np.asarray(Wh, dtype=np.float32))
    in_maps = [
        {
            "inputs": inputs[i * BC:(i + 1) * BC],
            "h_prev": h_prev[i * BC:(i + 1) * BC],
            "Wz": Wz,
            "Wr": Wr,
            "Wh": Wh,
        }
        for i in range(N_CORES)
    ]
    res = run_bass_kernel_spmd(
        nc, in_maps, core_ids=list(range(N_CORES)), trace=trace
    )
    out = np.concatenate([res.results[i]["out"] for i in range(N_CORES)], axis=0)
    return out, res


def kernel(inputs, h_prev, Wz, Wr, Wh):
    out, _ = run_sharded(inputs, h_prev, Wz, Wr, Wh, trace=False)
    return out


# revision 15
# speedup vs baseline: 1.0082x; 1.0082x over previous
"""Trainium2 Bass kernel for a custom GRU cell.

    x_h   = concat([inputs, h_prev], -1)            # [B, D+U]
    z     = sigmoid(x_h @ Wz)                       # [B, U]
    r     = sigmoid(x_h @ Wr)                       # [B, U]
    h_hat = tanh(concat([inputs, r * h_prev]) @ Wh) # [B, U]
    out   = z * h_prev + (1 - z) * h_hat

Data-parallel over 8 NeuronCores: batch sharded, weights replicated.

Per-core (B_c = 2048 rows, processed as 4 blocks of 512):
  - all matmuls in f32r (fp32 HIGH-half mode: ~bf16 speed, ~2^-13 precision)
  - x_h is transposed on the PE (f32r transpose-mode, 128x128 blocks) into
    feature-major k-tiles xh[k] [128, 512-batch], staged through PSUM
    and copied to SBUF by ScalarE
  - gate z batch-major:  psum[b,u]  = xh[k][:,bslice].T @ Wz[k]
  - gate r TRANSPOSED:   psum[u,b]  = Wr[k][:,uslice].T @ xh[k]
    so r*h_prev is computed transposed (rT * hT, where hT = xh[4+u]) with
    no extra transposes, and feeds gate h as lhsT directly
  - gate h batch-major, tanh/sigmoid on ScalarE straight out of PSUM,
    combine on VectorE, DMA out
  - emission is software-pipelined: gate-h of block i after z/r of i+1
"""

import sys

for _p in ("/opt/trn_rl_repo", "/root/.axon_site/_ro/trn_rl_repo"):
    if _p not in sys.path:
        sys.path.append(_p)

import numpy as np

B, D, U = 16384, 512, 512
K = D + U
N_CORES = 8
BC = B // N_CORES          # rows per core (2048)
BB = 512                   # batch-block rows
NB = BC // BB              # blocks per core (4)
KC = K // 128              # contraction chunks (8)


def build_gru_tile_kernel(tc, d_in, d_hp, d_wz, d_wr, d_wh, d_out, nb=NB):
    """Emit the GRU cell body into TileContext `tc`."""
    import contextlib

    from concourse import mybir
    from concourse.masks import make_identity

    f32 = mybir.dt.float32
    f32r = mybir.dt.float32r
    nc = tc.nc
    Sig = mybir.ActivationFunctionType.Sigmoid
    Tanh = mybir.ActivationFunctionType.Tanh

    est = contextlib.ExitStack()
    sing = est.enter_context(tc.tile_pool(name="sing", bufs=1))
    wpool = est.enter_context(tc.tile_pool(name="w", bufs=1))
    io = est.enter_context(tc.tile_pool(name="io", bufs=8))
    hpool = est.enter_context(tc.tile_pool(name="hpool", bufs=8))
    xhp = est.enter_context(tc.tile_pool(name="xhp", bufs=16))
    rhp = est.enter_context(tc.tile_pool(name="rhp", bufs=6))
    actp = est.enter_context(tc.tile_pool(name="act", bufs=6))
    tmpp = est.enter_context(tc.tile_pool(name="tmp", bufs=4))
    # PSUM: 3 transpose staging banks + 5 gate banks = 8/8
    # PSUM: 3 transpose staging banks + 5 gate banks = 8/8
    pst = est.enter_context(tc.tile_pool(name="pst", bufs=3, space="PSUM"))
    psg = est.enter_context(tc.tile_pool(name="psg", bufs=5, space="PSUM"))

    ident0 = sing.tile([128, 128], f32)
    make_identity(nc, ident0)
    identr = sing.tile([128, 128], f32r)
    nc.scalar.copy(identr[:], ident0[:])

    # ---- DMA schedule ----
    # The DMA engines drain instructions roughly in issue order, so load
    # block 0/1 activations first (unblocks the PE transposes ~12us in),
    # then stream the weights per-chunk (each z/r/h matmul only waits on
    # its own chunk), interleaved with the remaining blocks.
    pre_x = {}
    def load_x(bb):
        xin, hps = [], []
        for j in range(4):
            r0 = bb * BB + 128 * j
            x_j = io.tile([128, 512], f32r, tag="xin", name=f"x_{bb}_{j}")
            nc.sync.dma_start(x_j[:], d_in[r0:r0 + 128, :].bitcast(f32r))
            xin.append(x_j)
            h_j = hpool.tile([128, 512], f32r, tag="hp", name=f"h_{bb}_{j}")
            nc.sync.dma_start(h_j[:], d_hp[r0:r0 + 128, :].bitcast(f32r))
            hps.append(h_j)
        pre_x[bb] = (xin, hps)

    w_sb = {}
    def load_w(name, dram):
        t = wpool.tile([128, KC, 512], f32r, tag=name, name=name)
        for k in range(KC):
            nc.sync.dma_start(t[:, k, :], dram[128 * k:128 * (k + 1), :].bitcast(f32r))
        w_sb[name] = t

    load_x(0)
    load_w("wz", d_wz)
    load_w("wr", d_wr)
    if nb > 1:
        load_x(1)
    load_w("wh", d_wh)
    for bb in range(2, nb):
        load_x(bb)

    state = [None] * nb

    def phase_zr(bb):
        xin, hps = pre_x[bb]

        # ---- PE-transpose into feature-major k-tiles xh[k] [128, 512b] ----
        xh = [None] * KC
        xh_tiles = []

        def transpose_group(k):
            ps1 = pst.tile([128, 512], f32r, tag="pst", name=f"pst_{bb}_{k}")
            src = xin if k < 4 else hps
            kk = k % 4
            for j in range(4):
                nc.tensor.transpose(ps1[:, 128 * j:128 * (j + 1)],
                                    src[j][:, 128 * kk:128 * (kk + 1)], identr[:])
            sb1 = xhp.tile([128, 512], f32r, tag="xh", name=f"xh_{bb}_{k}")
            nc.scalar.copy(sb1[:], ps1[:])
            xh_tiles.append(sb1)
            xh[k] = sb1[:]

        for k in range(KC):
            transpose_group(k)

        # gate z, batch-major: ps[b,u] += xh[k][:,j].T @ Wz[k]
        zs = []
        for j in range(4):
            ps = psg.tile([128, 512], f32, tag="psg", name=f"psz_{bb}_{j}")
            for k in range(KC):
                nc.tensor.matmul(ps[:], xh[k][:, 128 * j:128 * (j + 1)],
                                 w_sb["wz"][:, k, :],
                                 start=(k == 0), stop=(k == KC - 1))
            z_j = actp.tile([128, 512], f32, tag="z", name=f"z_{bb}_{j}")
            nc.scalar.activation(z_j[:], ps[:], Sig)
            zs.append(z_j)

        # ---- gate r, transposed: ps[u,b] += Wr[k][:,u].T @ xh[k] ----
        rhT = []
        for u in range(4):
            ps = psg.tile([128, 512], f32, tag="psg", name=f"psr_{bb}_{u}")
            for k in range(KC):
                nc.tensor.matmul(ps[:], w_sb["wr"][:, k, 128 * u:128 * (u + 1)],
                                 xh[k], start=(k == 0), stop=(k == KC - 1))
            rT_u = actp.tile([128, 512], f32, tag="rT", name=f"rT_{bb}_{u}")
            nc.scalar.activation(rT_u[:], ps[:], Sig)
            # rhT[u] = rT[u] * h_prev.T[u]  (hT = xh[4+u]), f32r out
            rh_u = rhp.tile([128, 512], f32r, tag="rhT", name=f"rh_{bb}_{u}")
            nc.vector.tensor_mul(rh_u[:], rT_u[:], xh[4 + u].bitcast(f32))
            rhT.append(rh_u)

        state[bb] = (xh, xh_tiles, hps, zs, rhT)

    def phase_h(bb):
        xh, xh_tiles, hps, zs, rhT = state[bb]
        for j in range(4):
            ps = psg.tile([128, 512], f32, tag="psg", name=f"psh_{bb}_{j}")
            for k in range(KC):
                lhs = (xh[k][:, 128 * j:128 * (j + 1)] if k < 4
                       else rhT[k - 4][:, 128 * j:128 * (j + 1)])
                nc.tensor.matmul(ps[:], lhs, w_sb["wh"][:, k, :],
                                 start=(k == 0), stop=(k == KC - 1))
            hh = actp.tile([128, 512], f32, tag="hh", name=f"hh_{bb}_{j}")
            nc.scalar.activation(hh[:], ps[:], Tanh)

            # out = hh + z * (hp - hh)
            hp_f = hps[j][:].bitcast(f32)
            t = tmpp.tile([128, 512], f32, tag="tmp", name=f"t_{bb}_{j}")
            nc.vector.tensor_sub(t[:], hp_f, hh[:])
            t2 = tmpp.tile([128, 512], f32, tag="tmp", name=f"t2_{bb}_{j}")
            nc.vector.tensor_mul(t2[:], zs[j][:], t[:])
            out = tmpp.tile([128, 512], f32, tag="out", name=f"o_{bb}_{j}")
            nc.vector.tensor_add(out[:], t2[:], hh[:])
            r0 = bb * BB + 128 * j
            nc.sync.dma_start(d_out[r0:r0 + 128, :], out[:])
        state[bb] = None

    phase_zr(0)
    for bb in range(1, nb):
        phase_zr(bb)
        phase_h(bb - 1)
    phase_h(nb - 1)

    est.close()


_NC_CACHE = {}


def _build(nb=NB):
    if nb in _NC_CACHE:
        return _NC_CACHE[nb]
    import concourse.tile as tile
    from concourse import bacc, mybir

    f32 = mybir.dt.float32
    nc = bacc.Bacc("TRN2", target_bir_lowering=False, debug=False)
    d_in = nc.dram_tensor("inputs", [nb * BB, D], f32, kind="ExternalInput").ap()
    d_hp = nc.dram_tensor("h_prev", [nb * BB, U], f32, kind="ExternalInput").ap()
    d_wz = nc.dram_tensor("Wz", [K, U], f32, kind="ExternalInput").ap()
    d_wr = nc.dram_tensor("Wr", [K, U], f32, kind="ExternalInput").ap()
    d_wh = nc.dram_tensor("Wh", [K, U], f32, kind="ExternalInput").ap()
    d_out = nc.dram_tensor("out", [nb * BB, U], f32, kind="ExternalOutput").ap()

    with tile.TileContext(nc) as tc:
        build_gru_tile_kernel(tc, d_in, d_hp, d_wz, d_wr, d_wh, d_out, nb=nb)
    nc.compile()
    _NC_CACHE[nb] = nc
    return nc


def run_sharded(inputs, h_prev, Wz, Wr, Wh, trace=False):
    from concourse.bass_utils import run_bass_kernel_spmd

    nc = _build()
    inputs = np.ascontiguousarray(np.asarray(inputs, dtype=np.float32))
    h_prev = np.ascontiguousarray(np.asarray(h_prev, dtype=np.float32))
    Wz = np.ascontiguousarray(np.asarray(Wz, dtype=np.float32))
    Wr = np.ascontiguousarray(np.asarray(Wr, dtype=np.float32))
    Wh = np.ascontiguousarray(np.asarray(Wh, dtype=np.float32))
    in_maps = [
        {
            "inputs": inputs[i * BC:(i + 1) * BC],
            "h_prev": h_prev[i * BC:(i + 1) * BC],
            "Wz": Wz,
            "Wr": Wr,
            "Wh": Wh,
        }
        for i in range(N_CORES)
    ]
    res = run_bass_kernel_spmd(
        nc, in_maps, core_ids=list(range(N_CORES)), trace=trace
    )
    out = np.concatenate([res.results[i]["out"] for i in range(N_CORES)], axis=0)
    return out, res


def kernel(inputs, h_prev, Wz, Wr, Wh):
    out, _ = run_sharded(inputs, h_prev, Wz, Wr, Wh, trace=False)
    return out


# revision 17
# speedup vs baseline: 1.0403x; 1.0319x over previous
"""Trainium2 Bass kernel for a custom GRU cell.

    x_h   = concat([inputs, h_prev], -1)            # [B, D+U]
    z     = sigmoid(x_h @ Wz)                       # [B, U]
    r     = sigmoid(x_h @ Wr)                       # [B, U]
    h_hat = tanh(concat([inputs, r * h_prev]) @ Wh) # [B, U]
    out   = z * h_prev + (1 - z) * h_hat

Data-parallel over 8 NeuronCores: batch sharded, weights replicated.

Per-core (B_c = 2048 rows, processed as 4 blocks of 512):
  - all matmuls in f32r (fp32 HIGH-half mode: ~bf16 speed, ~2^-13 precision)
  - x_h is transposed on the PE (f32r transpose-mode, 128x128 blocks) into
    feature-major k-tiles xh[k] [128, 512-batch], staged through PSUM
    and copied to SBUF by ScalarE
  - gate z batch-major:  psum[b,u]  = xh[k][:,bslice].T @ Wz[k]
  - gate r TRANSPOSED:   psum[u,b]  = Wr[k][:,uslice].T @ xh[k]
    so r*h_prev is computed transposed (rT * hT, where hT = xh[4+u]) with
    no extra transposes, and feeds gate h as lhsT directly
  - gate h batch-major, tanh/sigmoid on ScalarE straight out of PSUM,
    combine on VectorE, DMA out
  - emission is software-pipelined: gate-h of block i after z/r of i+1
"""

import sys

for _p in ("/opt/trn_rl_repo", "/root/.axon_site/_ro/trn_rl_repo"):
    if _p not in sys.path:
        sys.path.append(_p)

import numpy as np

B, D, U = 16384, 512, 512
K = D + U
N_CORES = 8
BC = B // N_CORES          # rows per core (2048)
BB = 512                   # batch-block rows
NB = BC // BB              # blocks per core (4)
KC = K // 128              # contraction chunks (8)


def build_gru_tile_kernel(tc, d_in, d_hp, d_wz, d_wr, d_wh, d_out, nb=NB):
    """Emit the GRU cell body into TileContext `tc`."""
    import contextlib

    from concourse import mybir
    from concourse.masks import make_identity

    f32 = mybir.dt.float32
    f32r = mybir.dt.float32r
    nc = tc.nc
    Sig = mybir.ActivationFunctionType.Sigmoid
    Tanh = mybir.ActivationFunctionType.Tanh

    est = contextlib.ExitStack()
    sing = est.enter_context(tc.tile_pool(name="sing", bufs=1))
    wpool = est.enter_context(tc.tile_pool(name="w", bufs=1))
    io = est.enter_context(tc.tile_pool(name="io", bufs=8))
    hpool = est.enter_context(tc.tile_pool(name="hpool", bufs=8))
    xhp = est.enter_context(tc.tile_pool(name="xhp", bufs=16))
    rhp = est.enter_context(tc.tile_pool(name="rhp", bufs=6))
    actp = est.enter_context(tc.tile_pool(name="act", bufs=6))
    tmpp = est.enter_context(tc.tile_pool(name="tmp", bufs=4))
    # PSUM: 3 transpose staging banks + 5 gate banks = 8/8
    # PSUM: 3 transpose staging banks + 5 gate banks = 8/8
    pst = est.enter_context(tc.tile_pool(name="pst", bufs=3, space="PSUM"))
    psg = est.enter_context(tc.tile_pool(name="psg", bufs=5, space="PSUM"))

    ident0 = sing.tile([128, 128], f32)
    make_identity(nc, ident0)
    identr = sing.tile([128, 128], f32r)
    nc.scalar.copy(identr[:], ident0[:])

    # ---- DMA schedule ----
    # The DMA engines drain instructions roughly in issue order, so load
    # block 0/1 activations first (unblocks the PE transposes ~12us in),
    # then stream the weights per-chunk (each z/r/h matmul only waits on
    # its own chunk), interleaved with the remaining blocks.
    pre_x = {}
    def load_x(bb):
        xin, hps = [], []
        for j in range(4):
            r0 = bb * BB + 128 * j
            x_j = io.tile([128, 512], f32r, tag="xin", name=f"x_{bb}_{j}")
            nc.sync.dma_start(x_j[:], d_in[r0:r0 + 128, :].bitcast(f32r))
            xin.append(x_j)
            h_j = hpool.tile([128, 512], f32r, tag="hp", name=f"h_{bb}_{j}")
            nc.sync.dma_start(h_j[:], d_hp[r0:r0 + 128, :].bitcast(f32r))
            hps.append(h_j)
        pre_x[bb] = (xin, hps)

    w_sb = {}
    def load_w(name, dram):
        t = wpool.tile([128, KC, 512], f32r, tag=name, name=name)
        for k in range(KC):
            nc.sync.dma_start(t[:, k, :], dram[128 * k:128 * (k + 1), :].bitcast(f32r))
        w_sb[name] = t

    load_x(0)
    load_w("wz", d_wz)
    load_w("wr", d_wr)
    if nb > 1:
        load_x(1)
    load_w("wh", d_wh)
    for bb in range(2, nb):
        load_x(bb)

    state = [None] * nb

    def phase_zr(bb):
        xin, hps = pre_x[bb]

        # ---- PE-transpose into feature-major k-tiles xh[k] [128, 512b] ----
        xh = [None] * KC
        xh_tiles = []

        def transpose_group(k):
            ps1 = pst.tile([128, 512], f32r, tag="pst", name=f"pst_{bb}_{k}")
            src = xin if k < 4 else hps
            kk = k % 4
            for j in range(4):
                nc.tensor.transpose(ps1[:, 128 * j:128 * (j + 1)],
                                    src[j][:, 128 * kk:128 * (kk + 1)], identr[:])
            sb1 = xhp.tile([128, 512], f32r, tag="xh", name=f"xh_{bb}_{k}")
            nc.scalar.copy(sb1[:], ps1[:])
            xh_tiles.append(sb1)
            xh[k] = sb1[:]

        for k in range(KC):
            transpose_group(k)

        # gate z, batch-major: ps[b,u] += xh[k][:,j].T @ Wz[k]
        zs = []
        for j in range(4):
            ps = psg.tile([128, 512], f32, tag="psg", name=f"psz_{bb}_{j}")
            for k in range(KC):
                nc.tensor.matmul(ps[:], xh[k][:, 128 * j:128 * (j + 1)],
                                 w_sb["wz"][:, k, :],
                                 start=(k == 0), stop=(k == KC - 1))
            z_j = tmpp.tile([128, 512], f32, tag="tmp", name=f"z_{bb}_{j}")
            nc.scalar.activation(z_j[:], ps[:], Sig)
            # Precompute zc = 1 - z (ACT) and zh = z * h_prev (DVE) now, so
            # the post-tanh chain in phase_h is only two VectorE ops.
            zc_j = actp.tile([128, 512], f32, tag="zc", name=f"zc_{bb}_{j}")
            nc.scalar.activation(zc_j[:], z_j[:],
                                 mybir.ActivationFunctionType.Copy,
                                 bias=1.0, scale=-1.0)
            zh_j = actp.tile([128, 512], f32, tag="zh", name=f"zh_{bb}_{j}")
            nc.vector.tensor_mul(zh_j[:], z_j[:], hps[j][:].bitcast(f32))
            zs.append((zc_j, zh_j))

        # ---- gate r, transposed: ps[u,b] += Wr[k][:,u].T @ xh[k] ----
        rhT = []
        for u in range(4):
            ps = psg.tile([128, 512], f32, tag="psg", name=f"psr_{bb}_{u}")
            for k in range(KC):
                nc.tensor.matmul(ps[:], w_sb["wr"][:, k, 128 * u:128 * (u + 1)],
                                 xh[k], start=(k == 0), stop=(k == KC - 1))
            rT_u = actp.tile([128, 512], f32, tag="rT", name=f"rT_{bb}_{u}")
            nc.scalar.activation(rT_u[:], ps[:], Sig)
            # rhT[u] = rT[u] * h_prev.T[u]  (hT = xh[4+u]), f32r out
            rh_u = rhp.tile([128, 512], f32r, tag="rhT", name=f"rh_{bb}_{u}")
            nc.vector.tensor_mul(rh_u[:], rT_u[:], xh[4 + u].bitcast(f32))
            rhT.append(rh_u)

        state[bb] = (xh, xh_tiles, hps, zs, rhT)

    def phase_h(bb):
        xh, xh_tiles, hps, zs, rhT = state[bb]
        for j in range(4):
            ps = psg.tile([128, 512], f32, tag="psg", name=f"psh_{bb}_{j}")
            for k in range(KC):
                lhs = (xh[k][:, 128 * j:128 * (j + 1)] if k < 4
                       else rhT[k - 4][:, 128 * j:128 * (j + 1)])
                nc.tensor.matmul(ps[:], lhs, w_sb["wh"][:, k, :],
                                 start=(k == 0), stop=(k == KC - 1))
            hh = actp.tile([128, 512], f32, tag="hh", name=f"hh_{bb}_{j}")
            nc.scalar.activation(hh[:], ps[:], Tanh)

            # out = (1 - z) * hh + z * hp, with both z-terms precomputed
            zc_j, zh_j = zs[j]
            t2 = tmpp.tile([128, 512], f32, tag="tmp", name=f"t2_{bb}_{j}")
            nc.vector.tensor_mul(t2[:], zc_j[:], hh[:])
            out = tmpp.tile([128, 512], f32, tag="out", name=f"o_{bb}_{j}")
            nc.vector.tensor_add(out[:], t2[:], zh_j[:])
            r0 = bb * BB + 128 * j
            nc.sync.dma_start(d_out[r0:r0 + 128, :], out[:])
        state[bb] = None

    phase_zr(0)
    for bb in range(1, nb):
        phase_zr(bb)
        phase_h(bb - 1)
    phase_h(nb - 1)

    est.close()


_NC_CACHE = {}


def _build(nb=NB):
    if nb in _NC_CACHE:
        return _NC_CACHE[nb]
    import concourse.tile as tile
    from concourse import bacc, mybir

    f32 = mybir.dt.float32
    nc = bacc.Bacc("TRN2", target_bir_lowering=False, debug=False)
    d_in = nc.dram_tensor("inputs", [nb * BB, D], f32, kind="ExternalInput").ap()
    d_hp = nc.dram_tensor("h_prev", [nb * BB, U], f32, kind="ExternalInput").ap()
    d_wz = nc.dram_tensor("Wz", [K, U], f32, kind="ExternalInput").ap()
    d_wr = nc.dram_tensor("Wr", [K, U], f32, kind="ExternalInput").ap()
    d_wh = nc.dram_tensor("Wh", [K, U], f32, kind="ExternalInput").ap()
    d_out = nc.dram_tensor("out", [nb * BB, U], f32, kind="ExternalOutput").ap()

    with tile.TileContext(nc) as tc:
        build_gru_tile_kernel(tc, d_in, d_hp, d_wz, d_wr, d_wh, d_out, nb=nb)
    nc.compile()
    _NC_CACHE[nb] = nc
    return nc


def run_sharded(inputs, h_prev, Wz, Wr, Wh, trace=False):
    from concourse.bass_utils import run_bass_kernel_spmd

    nc = _build()
    inputs = np.ascontiguousarray(np.asarray(inputs, dtype=np.float32))
    h_prev = np.ascontiguousarray(np.asarray(h_prev, dtype=np.float32))
    Wz = np.ascontiguousarray(np.asarray(Wz, dtype=np.float32))
    Wr = np.ascontiguousarray(np.asarray(Wr, dtype=np.float32))
    Wh = np.ascontiguousarray(np.asarray(Wh, dtype=np.float32))
    in_maps = [
        {
            "inputs": inputs[i * BC:(i + 1) * BC],
            "h_prev": h_prev[i * BC:(i + 1) * BC],
            "Wz": Wz,
            "Wr": Wr,
            "Wh": Wh,
        }
        for i in range(N_CORES)
    ]
    res = run_bass_kernel_spmd(
        nc, in_maps, core_ids=list(range(N_CORES)), trace=trace
    )
    out = np.concatenate([res.results[i]["out"] for i in range(N_CORES)], axis=0)
    return out, res


def kernel(inputs, h_prev, Wz, Wr, Wh):
    out, _ = run_sharded(inputs, h_prev, Wz, Wr, Wh, trace=False)
    return out
